# revision 35
# baseline (speedup 1.0000x reference)
"""Multi-head attention (B=4, N=2048, D=768, H=12) on 8 TRN2 NeuronCores.

Sharding: 4 batches x 2 head-groups (6 heads each) = 8 cores, data-parallel
over batch and tensor-parallel over heads (column-sliced Wq/Wk/Wv, row-sliced
Wo). Each core computes, for its (batch b, head-group g):
    Qh = x_q @ Wq_g + bq_g ; Kh = x_k @ Wk_g ; Vh = x_v @ Wv_g
    y_partial = softmax(Qh Kh^T / sqrt(64)) Vh @ Wo_g          # [2048, 768]
Host gathers: out[b] = y_partial[2b] + y_partial[2b+1] + bo + bv @ Wo.
(bk provably cancels inside softmax; bv/bo fold into a constant row vector —
both exact identities, not approximations.)

Device program (per core, identical SPMD across the 8 cores):
 - Inputs are host-pre-transposed to feature-major x^T [768, 2048] bf16 so
   every matmul contracts over the SBUF partition dimension.
 - Q^T/K^T live zero-padded per head ([128, 6, 2048] bf16; head h occupies
   partitions 64*(h%2)..+64, the other 64 partitions are zeros), making all
   score matmuls uniform 128-contraction (the zero half contributes nothing
   and costs no extra PE cycles; no PE array-mode switches).
 - V is token-major [128, 16, 6, 65] bf16 with a ones column, so each PV
   matmul also emits the softmax denominator in output column 64 for free.
 - Scores S^T[k, q] accumulate in PSUM [128, 1024] (two 512-wide matmuls per
   bank pair); ScalarE applies exp(0.125 * s) straight out of PSUM into bf16
   P^T tiles (max |0.125*s| ~ 10 << 88, so no max-subtraction is needed).
 - PV uses P^T chunks as the stationary operand so attention output lands
   q-major and softmax normalization is one reciprocal + per-partition scale.
 - The normalized [128, 64] tile is PE-transposed back to feature-major for
   the row-parallel output projection; y_partial leaves as fp32.
 - Emission order (= Tile scheduler priority) pipelines: stage x -> V & j0
   projections -> head-pair 0 attention -> j1 -> pair 1 -> j2 -> pair 2 with
   the output projection inlined per query block. ScalarE's ~200us of exp is
   the modeled bottleneck; TimelineSim predicts ~290us/core end-to-end.

Infrastructure notes:
 - The walrus build staged here accepts at most ONE sync wait per
   instruction; _split_multi_waits() rewrites Tile's schedule accordingly.
 - No NTFF profiling hook exists under this axon client, so test.py reports
   steady-state wall-clock (which includes ~5.5ms of tunnel dispatch/transfer
   per call) plus the TimelineSim estimate.
"""

import numpy as np
import ml_dtypes

import bass_rust
import concourse.bass as bass
import concourse.mybir as mybir
import concourse.tile as tile
from concourse.masks import make_identity

BF16 = mybir.dt.bfloat16
F32 = mybir.dt.float32
NPBF16 = ml_dtypes.bfloat16

B, N, D = 4, 2048, 768
H, DK = 12, 64
HG = 6            # heads per core
DG = HG * DK      # 384 features per head-group
KC = D // 128     # 6 contraction chunks for projections
TB = N // 512     # 4 token blocks of 512
KT = N // 128     # 16 key chunks of 128
QB = N // 512     # 4 query blocks of 512


def _split_multi_waits(nc: bass.Bass) -> int:
    """The walrus build staged here accepts at most ONE sync wait per TPB
    instruction; Tile's scheduler freely attaches several. Split: hoist all
    but the last wait of an instruction onto same-engine NoOps inserted
    immediately before it (same per-engine program order => same semantics).
    Applies to DMA pseudo-instructions too (their issuing engine stalls)."""
    n_split = 0
    n_new = 0
    for f in nc.m.functions:
        for bb in f.blocks:
            insts = list(bb.instructions)
            needs = False
            for inst in insts:
                si = inst.sync_info
                if si is not None and len(si.on_wait) > 1:
                    needs = True
                    break
            if not needs:
                continue
            new_list = []
            for inst in insts:
                si = inst.sync_info
                if si is not None and len(si.on_wait) > 1:
                    waits = list(si.on_wait)
                    for w in waits[:-1]:
                        nop = bass_rust.InstNoOp(
                            name=f"WSPLIT-{n_new}", ins=[], outs=[]
                        )
                        n_new += 1
                        nop.engine = inst.engine
                        nop.sync_info = bass_rust.SyncInfo(
                            on_wait=[w], on_update=[]
                        )
                        new_list.append(nop)
                    inst.sync_info = bass_rust.SyncInfo(
                        on_wait=[waits[-1]], on_update=list(si.on_update)
                    )
                    n_split += 1
                new_list.append(inst)
            bb.instructions = new_list
    return n_split


def build_nc(reps: int = 1, ablate: frozenset = frozenset()) -> bass.Bass:
    """Build the per-core program. reps>1 wraps the whole body in an
    on-device For_i loop (timing only; broken on this walrus). `ablate`
    (timing probes only, wrong results): "exp" skips the exp activations,
    "att" skips the whole attention phase, "tr" skips transpose+copy,
    "pv" skips PV/normalize/transpose."""
    nc = bass.Bass()

    xqT = nc.dram_tensor("xqT", [D, N], BF16, kind="ExternalInput")
    xkT = nc.dram_tensor("xkT", [D, N], BF16, kind="ExternalInput")
    xvT = nc.dram_tensor("xvT", [D, N], BF16, kind="ExternalInput")
    wq = nc.dram_tensor("wq", [D, DG], BF16, kind="ExternalInput")
    wk = nc.dram_tensor("wk", [D, DG], BF16, kind="ExternalInput")
    wv = nc.dram_tensor("wv", [D, DG], BF16, kind="ExternalInput")
    wo = nc.dram_tensor("wo", [DG, D], BF16, kind="ExternalInput")
    bq = nc.dram_tensor("bq", [DG], F32, kind="ExternalInput")
    y = nc.dram_tensor("y", [N, D], F32, kind="ExternalOutput")

    with tile.TileContext(nc) as tc:
        if ablate:
            tc.race_detector_enabled = False
        from contextlib import ExitStack

        with ExitStack() as ctx:
            const_pool = ctx.enter_context(tc.tile_pool(name="const", bufs=1))
            wpool = ctx.enter_context(tc.tile_pool(name="wpool", bufs=1))
            qkv_pool = ctx.enter_context(tc.tile_pool(name="qkv", bufs=1))
            xs_pool = ctx.enter_context(tc.tile_pool(name="xs", bufs=2))
            pt_pool = ctx.enter_context(tc.tile_pool(name="pt", bufs=3))
            small_pool = ctx.enter_context(tc.tile_pool(name="small", bufs=6))
            y_pool = ctx.enter_context(tc.tile_pool(name="ysb", bufs=2))

            loop_ctx = tc.For_i(0, reps, 1) if reps > 1 else None
            if loop_ctx is not None:
                ctx.enter_context(loop_ctx)

            ident = const_pool.tile([128, 128], BF16)
            make_identity(nc, ident)
            bq_sb = const_pool.tile([128, DG // 128], F32)
            nc.sync.dma_start(out=bq_sb, in_=bq.rearrange("(c p) -> p c", p=128))

            wq_sb = wpool.tile([128, KC, DG], BF16)
            wk_sb = wpool.tile([128, KC, DG], BF16)
            wv_sb = wpool.tile([128, KC, DG], BF16)
            for w_sb, w_dram in ((wq_sb, wq), (wk_sb, wk), (wv_sb, wv)):
                for c in range(KC):
                    nc.sync.dma_start(
                        out=w_sb[:, c, :], in_=w_dram[c * 128 : (c + 1) * 128, :]
                    )
            wo_sb = wpool.tile([128, DG // 128, D], BF16)
            for c in range(DG // 128):
                nc.sync.dma_start(
                    out=wo_sb[:, c, :], in_=wo[c * 128 : (c + 1) * 128, :]
                )

            # Persistent activations.
            qt_sb = qkv_pool.tile([128, HG, N], BF16)   # Q^T, zero-padded per head
            kt_sb = qkv_pool.tile([128, HG, N], BF16)   # K^T, zero-padded per head
            v_sb = qkv_pool.tile([128, KT, HG, DK + 1], BF16)  # V + ones column
            at_sb = qkv_pool.tile([128, DG // 128, N], BF16)   # attention out^T

            # Zero the unused halves of Q^T/K^T, set V's ones column.
            for h in range(HG):
                zlo = 64 * ((h + 1) % 2)
                nc.gpsimd.memset(qt_sb[zlo : zlo + 64, h, :], 0.0)
                nc.gpsimd.memset(kt_sb[zlo : zlo + 64, h, :], 0.0)
            nc.gpsimd.memset(v_sb[:, :, :, DK : DK + 1], 1.0)

            # PSUM pools, 8 banks total: s 2x2, o 2, t 1, ymm 1. V/j0/yproj
            # psums share the o slots (temporally disjoint from PV's use);
            # j1/j2 projections overlap attention so they get their own slot.
            ps_s = ctx.enter_context(tc.tile_pool(name="pss", bufs=2, space="PSUM"))
            ps_o = ctx.enter_context(tc.tile_pool(name="pso", bufs=2, space="PSUM"))
            ps_t = ctx.enter_context(tc.tile_pool(name="pst", bufs=1, space="PSUM"))
            ps_y = ctx.enter_context(tc.tile_pool(name="psy", bufs=1, space="PSUM"))
            ps_mm = ps_y
            if True:
                # Stage x_q^T / x_k^T whole; stream x_v^T in 512-token blocks
                # on the second HWDGE engine (ScalarE) in parallel.
                xq_sb = qkv_pool.tile([128, KC, N], BF16, tag="xq")
                xk_sb = qkv_pool.tile([128, KC, N], BF16, tag="xk")

                def stage_x(x_sb, x_dram, tb):
                    for c in range(KC):
                        nc.sync.dma_start(
                            out=x_sb[:, c, tb * 512 : (tb + 1) * 512],
                            in_=x_dram[c * 128 : (c + 1) * 128, tb * 512 : (tb + 1) * 512],
                        )

                def emit_vproj(blocks):
                    # V projection (token-major): V[tok,f] = sum_c xvT[c,:]^T wv[c,:]
                    # (psum shares the o-pool slots: V finishes before first PV)
                    for ktb in blocks:
                        xv_t = xs_pool.tile([128, KC, 512], BF16, tag="xv")
                        for c in range(KC):
                            nc.scalar.dma_start(
                                out=xv_t[:, c, :],
                                in_=xvT[c * 128 : (c + 1) * 128, ktb * 512 : (ktb + 1) * 512],
                            )
                        for kt4 in range(4):
                            kt = ktb * 4 + kt4
                            psum = ps_o.tile([128, 512], F32, tag="o")
                            for c in range(KC):
                                nc.tensor.matmul(
                                    psum[:, :DG],
                                    lhsT=xv_t[:, c, kt4 * 128 : (kt4 + 1) * 128],
                                    rhs=wv_sb[:, c, :],
                                    start=(c == 0),
                                    stop=(c == KC - 1),
                                )
                            nc.vector.tensor_copy(
                                v_sb[:, kt, :, 0:DK],
                                psum[:, :DG].rearrange("p (h d) -> p h d", h=HG),
                            )

                def emit_qkproj(j, tbs=None):
                    # j0 precedes all PV work -> may use the o-pool slots too
                    for tb in (range(TB) if tbs is None else tbs):
                        if j == 0:
                            psum = ps_o.tile([128, 512], F32, tag="o")
                        else:
                            psum = ps_mm.tile([128, 512], F32, tag="ymm")
                        for c in range(KC):
                            nc.tensor.matmul(
                                psum,
                                lhsT=wk_sb[:, c, j * 128 : (j + 1) * 128],
                                rhs=xk_sb[:, c, tb * 512 : (tb + 1) * 512],
                                start=(c == 0),
                                stop=(c == KC - 1),
                            )
                        nc.vector.tensor_copy(
                            kt_sb[0:64, 2 * j, tb * 512 : (tb + 1) * 512],
                            psum[0:64, :],
                        )
                        nc.vector.tensor_copy(
                            kt_sb[64:128, 2 * j + 1, tb * 512 : (tb + 1) * 512],
                            psum[64:128, :],
                        )
                        if j == 0:
                            psum = ps_o.tile([128, 512], F32, tag="o")
                        else:
                            psum = ps_mm.tile([128, 512], F32, tag="ymm")
                        for c in range(KC):
                            nc.tensor.matmul(
                                psum,
                                lhsT=wq_sb[:, c, j * 128 : (j + 1) * 128],
                                rhs=xq_sb[:, c, tb * 512 : (tb + 1) * 512],
                                start=(c == 0),
                                stop=(c == KC - 1),
                            )
                        # heads 2j (partitions 0:64) and 2j+1 (partitions 64:128)
                        nc.vector.tensor_scalar_add(
                            qt_sb[0:64, 2 * j, tb * 512 : (tb + 1) * 512],
                            psum[0:64, :],
                            bq_sb[0:64, j : j + 1],
                        )
                        nc.vector.tensor_scalar_add(
                            qt_sb[64:128, 2 * j + 1, tb * 512 : (tb + 1) * 512],
                            psum[64:128, :],
                            bq_sb[64:128, j : j + 1],
                        )

                # K staged first (scores need all of K^T but only one query
                # block), then Q; V and j0 projections fill PE meanwhile.
                for tb in range(TB):
                    stage_x(xk_sb, xkT, tb)
                for tb in range(TB):
                    stage_x(xq_sb, xqT, tb)
                emit_vproj(range(4))
                emit_qkproj(0)

            # --- attention
            if ablate:
                nc.gpsimd.memset(at_sb[:, :, :], 0.5)
                nc.gpsimd.memset(v_sb[:, :, :, :], 0.5)
            def emit_yproj(qt):
                y_sb = y_pool.tile([128, D], F32, tag="y")
                for nb in range(2):
                    y_psum = ps_o.tile([128, DG], F32, tag="o")
                    for fc in range(DG // 128):
                        nc.tensor.matmul(
                            y_psum,
                            lhsT=at_sb[:, fc, qt * 128 : (qt + 1) * 128],
                            rhs=wo_sb[:, fc, nb * DG : (nb + 1) * DG],
                            start=(fc == 0),
                            stop=(fc == DG // 128 - 1),
                        )
                    nc.vector.tensor_copy(y_sb[:, nb * DG : (nb + 1) * DG], y_psum)
                nc.sync.dma_start(out=y[qt * 128 : (qt + 1) * 128, :], in_=y_sb)

            pair_range = [] if "att" in ablate else range(3)
            for hp in pair_range:
                if hp > 0:
                    emit_qkproj(hp)
                for qb in range(QB):
                  for h in (2 * hp, 2 * hp + 1):
                    pt_t = pt_pool.tile([128, KT, 512], BF16, tag="pt")
                    for kt2 in range(KT // 2):
                        s_psum = ps_s.tile([128, 1024], F32, tag="s")
                        for half in range(2):
                            kt = 2 * kt2 + half
                            nc.tensor.matmul(
                                s_psum[:, half * 512 : (half + 1) * 512],
                                lhsT=kt_sb[:, h, kt * 128 : (kt + 1) * 128],
                                rhs=qt_sb[:, h, qb * 512 : (qb + 1) * 512],
                                start=True,
                                stop=True,
                            )
                        if "exp" not in ablate:
                            nc.scalar.activation(
                                pt_t[:, 2 * kt2 : 2 * kt2 + 2, :],
                                s_psum.rearrange("p (k f) -> p k f", k=2),
                                mybir.ActivationFunctionType.Exp,
                                scale=0.125,
                            )
                        else:
                            nc.gpsimd.memset(pt_t[:, 2 * kt2, 0:1], 0.5)
                    plo = 64 * (h % 2)
                    pv_range = [] if "pv" in ablate else range(4)
                    for qt in pv_range:
                        o_psum = ps_o.tile([128, DK + 1], F32, tag="o")
                        for kt in range(KT):
                            nc.tensor.matmul(
                                o_psum,
                                lhsT=pt_t[:, kt, qt * 128 : (qt + 1) * 128],
                                rhs=v_sb[:, kt, h, :],
                                start=(kt == 0),
                                stop=(kt == KT - 1),
                            )
                        recip = small_pool.tile([128, 1], F32, tag="recip")
                        nc.vector.reciprocal(recip, o_psum[:, DK : DK + 1])
                        attn = small_pool.tile([128, DK], BF16, tag="attn")
                        nc.vector.tensor_scalar_mul(attn, o_psum[:, 0:DK], recip)
                        if "tr" not in ablate:
                            t_psum = ps_t.tile([128, 128], BF16, tag="t")
                            nc.tensor.transpose(
                                t_psum[plo : plo + 64, :], attn, ident
                            )
                            nc.vector.tensor_copy(
                                at_sb[
                                    plo : plo + 64,
                                    h // 2,
                                    qb * 512 + qt * 128 : qb * 512 + (qt + 1) * 128,
                                ],
                                t_psum[plo : plo + 64, :],
                            )
                    if h == HG - 1 and "tr" not in ablate:
                        for qt4 in range(4):
                            emit_yproj(qb * 4 + qt4)

    _split_multi_waits(nc)
    return nc


_NC_CACHE: list = []


def _get_nc() -> bass.Bass:
    if not _NC_CACHE:
        _NC_CACHE.append(build_nc())
    return _NC_CACHE[0]


_RUNNER_CACHE: dict = {}


def _get_runner():
    """Compile once per process; later kernel() calls reuse the jitted
    executable (jax jit cache) and only re-upload inputs."""
    if _RUNNER_CACHE:
        return _RUNNER_CACHE["r"]
    import jax
    from jax.sharding import Mesh, PartitionSpec
    from jax.experimental.shard_map import shard_map
    from concourse import bass2jax

    nc = _get_nc()
    bass2jax.install_neuronx_cc_hook()
    partition_name = nc.partition_id_tensor.name if nc.partition_id_tensor else None
    in_names, out_names, out_avals, zero_outs = [], [], [], []
    for alloc in nc.m.functions[0].allocations:
        if not isinstance(alloc, mybir.MemoryLocationSet):
            continue
        name = alloc.memorylocations[0].name
        if alloc.kind == "ExternalInput":
            if name != partition_name:
                in_names.append(name)
        elif alloc.kind == "ExternalOutput":
            out_names.append(name)
            shape = tuple(alloc.tensor_shape)
            dtype = mybir.dt.np(alloc.dtype)
            out_avals.append(jax.core.ShapedArray(shape, dtype))
            zero_outs.append(np.zeros(shape, dtype))
    n_params = len(in_names)
    all_in_names = list(in_names) + list(out_names)
    if partition_name is not None:
        all_in_names.append(partition_name)

    def _body(*args):
        operands = list(args)
        if partition_name is not None:
            operands.append(bass2jax.partition_id_tensor())
        outs = bass2jax._bass_exec_p.bind(
            *operands,
            out_avals=tuple(out_avals),
            in_names=tuple(all_in_names),
            out_names=tuple(out_names),
            lowering_input_output_aliases=(),
            sim_require_finite=True,
            sim_require_nnan=True,
            nc=nc,
        )
        return tuple(outs)

    n_cores = 8
    devices = jax.devices()[:n_cores]
    mesh = Mesh(np.asarray(devices), ("core",))
    in_specs = (PartitionSpec("core"),) * (n_params + len(out_avals))
    out_specs = (PartitionSpec("core"),) * len(out_names)
    sharded = jax.jit(
        shard_map(_body, mesh=mesh, in_specs=in_specs, out_specs=out_specs,
                  check_rep=False),
        keep_unused=True,
    )
    concat_zeros = [
        np.zeros((n_cores * z.shape[0], *z.shape[1:]), z.dtype) for z in zero_outs
    ]

    import zlib

    dev_cache: dict = {}

    def run(in_maps):
        per_core = [[np.asarray(m[nm]) for nm in in_names] for m in in_maps]
        concat_in = [
            np.concatenate([per_core[c][i] for c in range(n_cores)], axis=0)
            for i in range(n_params)
        ]
        # Re-upload through the tunnel only when input content changes.
        key = tuple(
            (a.shape, str(a.dtype), zlib.adler32(a.tobytes())) for a in concat_in
        )
        if dev_cache.get("key") != key:
            dev_cache["args"] = [jax.device_put(a) for a in concat_in] + [
                jax.device_put(z) for z in concat_zeros
            ]
            dev_cache["key"] = key
        out = sharded(*dev_cache["args"])
        arrs = [np.asarray(x) for x in out]
        return [
            {
                nm: arrs[i].reshape(n_cores, *out_avals[i].shape)[c]
                for i, nm in enumerate(out_names)
            }
            for c in range(n_cores)
        ]

    _RUNNER_CACHE["r"] = run
    return run


def make_in_maps(q, k, v, Wq, bq, Wk, bk, Wv, bv, Wo, bo):
    q = np.asarray(q, np.float32)
    k = np.asarray(k, np.float32)
    v = np.asarray(v, np.float32)
    Wq = np.asarray(Wq, np.float32)
    Wk = np.asarray(Wk, np.float32)
    Wv = np.asarray(Wv, np.float32)
    Wo = np.asarray(Wo, np.float32)
    bq = np.asarray(bq, np.float32)

    xT = {}
    for b in range(B):
        xT[b] = (
            np.ascontiguousarray(q[b].T).astype(NPBF16),
            np.ascontiguousarray(k[b].T).astype(NPBF16),
            np.ascontiguousarray(v[b].T).astype(NPBF16),
        )
    wslice = {}
    for g in range(2):
        sl = slice(g * DG, (g + 1) * DG)
        wslice[g] = (
            np.ascontiguousarray(Wq[:, sl]).astype(NPBF16),
            np.ascontiguousarray(Wk[:, sl]).astype(NPBF16),
            np.ascontiguousarray(Wv[:, sl]).astype(NPBF16),
            np.ascontiguousarray(Wo[sl, :]).astype(NPBF16),
            np.ascontiguousarray(bq[sl]),
        )
    in_maps = []
    for c in range(8):
        b, g = c // 2, c % 2
        qT, kT, vT = xT[b]
        wq_g, wk_g, wv_g, wo_g, bq_g = wslice[g]
        in_maps.append(
            {
                "xqT": qT, "xkT": kT, "xvT": vT,
                "wq": wq_g, "wk": wk_g, "wv": wv_g, "wo": wo_g, "bq": bq_g,
            }
        )
    return in_maps


def gather(results, bv, bo, Wo):
    bv = np.asarray(bv, np.float32)
    bo = np.asarray(bo, np.float32)
    Wo = np.asarray(Wo, np.float32)
    const_row = bo + bv @ Wo  # [768]
    out = np.empty((B, N, D), np.float32)
    for b in range(B):
        out[b] = results[2 * b]["y"] + results[2 * b + 1]["y"] + const_row
    return out


def kernel(q, k, v, Wq, bq, Wk, bk, Wv, bv, Wo, bo):
    in_maps = make_in_maps(q, k, v, Wq, bq, Wk, bk, Wv, bv, Wo, bo)
    results = _get_runner()(in_maps)
    return gather(results, bv, bo, Wo)


# revision 41
# speedup vs baseline: 1.1211x; 1.1211x over previous
"""Multi-head attention (B=4, N=2048, D=768, H=12) on 8 TRN2 NeuronCores.

Sharding: 4 batches x 2 head-groups (6 heads each) = 8 cores, data-parallel
over batch and tensor-parallel over heads (column-sliced Wq/Wk/Wv, row-sliced
Wo). Each core computes, for its (batch b, head-group g):
    Qh = x_q @ Wq_g + bq_g ; Kh = x_k @ Wk_g ; Vh = x_v @ Wv_g
    y_partial = softmax(Qh Kh^T / sqrt(64)) Vh @ Wo_g          # [2048, 768]
Host gathers: out[b] = y_partial[2b] + y_partial[2b+1] + bo + bv @ Wo.
(bk provably cancels inside softmax; bv/bo fold into a constant row vector —
both exact identities, not approximations.)

Device program (per core, identical SPMD across the 8 cores):
 - Inputs are host-pre-transposed to feature-major x^T [768, 2048] bf16 so
   every matmul contracts over the SBUF partition dimension.
 - Q^T/K^T live zero-padded per head ([128, 6, 2048] bf16; head h occupies
   partitions 64*(h%2)..+64, the other 64 partitions are zeros), making all
   score matmuls uniform 128-contraction (the zero half contributes nothing
   and costs no extra PE cycles; no PE array-mode switches).
 - V is token-major [128, 16, 6, 65] bf16 with a ones column, so each PV
   matmul also emits the softmax denominator in output column 64 for free.
 - Scores S^T[k, q] accumulate in PSUM [128, 1024] (two 512-wide matmuls per
   bank pair); ScalarE applies exp(0.125 * s) straight out of PSUM into bf16
   P^T tiles (max |0.125*s| ~ 10 << 88, so no max-subtraction is needed).
 - PV uses P^T chunks as the stationary operand so attention output lands
   q-major and softmax normalization is one reciprocal + per-partition scale.
 - The normalized [128, 64] tile is PE-transposed back to feature-major for
   the row-parallel output projection; y_partial leaves as fp32.
 - Emission order (= Tile scheduler priority) pipelines: stage x -> V & j0
   projections -> head-pair 0 attention -> j1 -> pair 1 -> j2 -> pair 2 with
   the output projection inlined per query block. ScalarE's ~200us of exp is
   the modeled bottleneck; TimelineSim predicts ~290us/core end-to-end.

Infrastructure notes:
 - The walrus build staged here accepts at most ONE sync wait per
   instruction; _split_multi_waits() rewrites Tile's schedule accordingly.
 - No NTFF profiling hook exists under this axon client, so test.py reports
   steady-state wall-clock (which includes ~5.5ms of tunnel dispatch/transfer
   per call) plus the TimelineSim estimate.
"""

import numpy as np
import ml_dtypes

import bass_rust
import concourse.bass as bass
import concourse.mybir as mybir
import concourse.tile as tile
from concourse.masks import make_identity

BF16 = mybir.dt.bfloat16
F32 = mybir.dt.float32
NPBF16 = ml_dtypes.bfloat16

B, N, D = 4, 2048, 768
H, DK = 12, 64
HG = 6            # heads per core
DG = HG * DK      # 384 features per head-group
KC = D // 128     # 6 contraction chunks for projections
TB = N // 512     # 4 token blocks of 512
KT = N // 128     # 16 key chunks of 128
QB = N // 512     # 4 query blocks of 512


def _split_multi_waits(nc: bass.Bass) -> int:
    """The walrus build staged here accepts at most ONE sync wait per TPB
    instruction; Tile's scheduler freely attaches several. Split: hoist all
    but the last wait of an instruction onto same-engine NoOps inserted
    immediately before it (same per-engine program order => same semantics).
    Applies to DMA pseudo-instructions too (their issuing engine stalls)."""
    n_split = 0
    n_new = 0
    for f in nc.m.functions:
        for bb in f.blocks:
            insts = list(bb.instructions)
            needs = False
            for inst in insts:
                si = inst.sync_info
                if si is not None and len(si.on_wait) > 1:
                    needs = True
                    break
            if not needs:
                continue
            new_list = []
            for inst in insts:
                si = inst.sync_info
                if si is not None and len(si.on_wait) > 1:
                    waits = list(si.on_wait)
                    for w in waits[:-1]:
                        nop = bass_rust.InstNoOp(
                            name=f"WSPLIT-{n_new}", ins=[], outs=[]
                        )
                        n_new += 1
                        nop.engine = inst.engine
                        nop.sync_info = bass_rust.SyncInfo(
                            on_wait=[w], on_update=[]
                        )
                        new_list.append(nop)
                    inst.sync_info = bass_rust.SyncInfo(
                        on_wait=[waits[-1]], on_update=list(si.on_update)
                    )
                    n_split += 1
                new_list.append(inst)
            bb.instructions = new_list
    return n_split


def build_nc(reps: int = 1, ablate: frozenset = frozenset()) -> bass.Bass:
    """Build the per-core program. reps>1 wraps the whole body in an
    on-device For_i loop (timing only; broken on this walrus). `ablate`
    (timing probes only, wrong results): "exp" skips the exp activations,
    "att" skips the whole attention phase, "tr" skips transpose+copy,
    "pv" skips PV/normalize/transpose."""
    nc = bass.Bass()

    xqT = nc.dram_tensor("xqT", [D, N], BF16, kind="ExternalInput")
    xkT = nc.dram_tensor("xkT", [D, N], BF16, kind="ExternalInput")
    xvT = nc.dram_tensor("xvT", [D, N], BF16, kind="ExternalInput")
    wq = nc.dram_tensor("wq", [D, DG], BF16, kind="ExternalInput")
    wk = nc.dram_tensor("wk", [D, DG], BF16, kind="ExternalInput")
    wv = nc.dram_tensor("wv", [D, DG], BF16, kind="ExternalInput")
    wo = nc.dram_tensor("wo", [DG, D], BF16, kind="ExternalInput")
    bq = nc.dram_tensor("bq", [DG], F32, kind="ExternalInput")
    y = nc.dram_tensor("y", [N, D], F32, kind="ExternalOutput")

    with tile.TileContext(nc) as tc:
        if ablate:
            tc.race_detector_enabled = False
        from contextlib import ExitStack

        with ExitStack() as ctx:
            const_pool = ctx.enter_context(tc.tile_pool(name="const", bufs=1))
            wpool = ctx.enter_context(tc.tile_pool(name="wpool", bufs=1))
            qkv_pool = ctx.enter_context(tc.tile_pool(name="qkv", bufs=1))
            xs_pool = ctx.enter_context(tc.tile_pool(name="xs", bufs=2))
            pt_pool = ctx.enter_context(tc.tile_pool(name="pt", bufs=3))
            small_pool = ctx.enter_context(tc.tile_pool(name="small", bufs=6))
            y_pool = ctx.enter_context(tc.tile_pool(name="ysb", bufs=2))

            loop_ctx = tc.For_i(0, reps, 1) if reps > 1 else None
            if loop_ctx is not None:
                ctx.enter_context(loop_ctx)

            ident = const_pool.tile([128, 128], BF16)
            make_identity(nc, ident)
            bq_sb = const_pool.tile([128, DG // 128], F32)
            nc.sync.dma_start(out=bq_sb, in_=bq.rearrange("(c p) -> p c", p=128))

            wq_sb = wpool.tile([128, KC, DG], BF16)
            wk_sb = wpool.tile([128, KC, DG], BF16)
            wv_sb = wpool.tile([128, KC, DG], BF16)
            for w_sb, w_dram in ((wq_sb, wq), (wk_sb, wk), (wv_sb, wv)):
                nc.sync.dma_start(
                    out=w_sb, in_=w_dram.rearrange("(c p) f -> p c f", p=128)
                )
            wo_sb = wpool.tile([128, DG // 128, D], BF16)
            nc.sync.dma_start(
                out=wo_sb, in_=wo.rearrange("(c p) f -> p c f", p=128)
            )

            # Persistent activations.
            qt_sb = qkv_pool.tile([128, HG, N], BF16)   # Q^T, zero-padded per head
            kt_sb = qkv_pool.tile([128, HG, N], BF16)   # K^T, zero-padded per head
            v_sb = qkv_pool.tile([128, KT, HG, DK + 1], BF16)  # V + ones column
            at_sb = qkv_pool.tile([128, DG // 128, N], BF16)   # attention out^T

            # Zero the unused halves of Q^T/K^T, set V's ones column.
            for h in range(HG):
                zlo = 64 * ((h + 1) % 2)
                nc.gpsimd.memset(qt_sb[zlo : zlo + 64, h, :], 0.0)
                nc.gpsimd.memset(kt_sb[zlo : zlo + 64, h, :], 0.0)
            nc.gpsimd.memset(v_sb[:, :, :, DK : DK + 1], 1.0)

            # PSUM pools, 8 banks total: s 2x2, o 2, t 1, ymm 1. V/j0/yproj
            # psums share the o slots (temporally disjoint from PV's use);
            # j1/j2 projections overlap attention so they get their own slot.
            ps_s = ctx.enter_context(tc.tile_pool(name="pss", bufs=2, space="PSUM"))
            ps_o = ctx.enter_context(tc.tile_pool(name="pso", bufs=2, space="PSUM"))
            ps_t = ctx.enter_context(tc.tile_pool(name="pst", bufs=1, space="PSUM"))
            ps_y = ctx.enter_context(tc.tile_pool(name="psy", bufs=1, space="PSUM"))
            ps_mm = ps_y
            if True:
                # Stage x_q^T / x_k^T whole; stream x_v^T in 512-token blocks
                # on the second HWDGE engine (ScalarE) in parallel.
                xq_sb = qkv_pool.tile([128, KC, N], BF16, tag="xq")
                xk_sb = qkv_pool.tile([128, KC, N], BF16, tag="xk")

                def stage_x(x_sb, x_dram, tb, eng):
                    eng.dma_start(
                        out=x_sb,
                        in_=x_dram.rearrange("(c p) t -> p c t", p=128),
                    )

                def emit_vproj(blocks):
                    # V projection (token-major): V[tok,f] = sum_c xvT[c,:]^T wv[c,:]
                    # (psum shares the o-pool slots: V finishes before first PV)
                    for ktb in blocks:
                        xv_t = xs_pool.tile([128, KC, 512], BF16, tag="xv")
                        nc.scalar.dma_start(
                            out=xv_t,
                            in_=xvT.rearrange("(c p) t -> p c t", p=128)[
                                :, :, ktb * 512 : (ktb + 1) * 512
                            ],
                        )
                        for kt4 in range(4):
                            kt = ktb * 4 + kt4
                            psum = ps_o.tile([128, 512], F32, tag="o")
                            for c in range(KC):
                                nc.tensor.matmul(
                                    psum[:, :DG],
                                    lhsT=xv_t[:, c, kt4 * 128 : (kt4 + 1) * 128],
                                    rhs=wv_sb[:, c, :],
                                    start=(c == 0),
                                    stop=(c == KC - 1),
                                )
                            nc.vector.tensor_copy(
                                v_sb[:, kt, :, 0:DK],
                                psum[:, :DG].rearrange("p (h d) -> p h d", h=HG),
                            )

                def emit_qkproj(j, tbs=None):
                    # j0 precedes all PV work -> may use the o-pool slots too
                    for tb in (range(TB) if tbs is None else tbs):
                        if j == 0:
                            psum = ps_o.tile([128, 512], F32, tag="o")
                        else:
                            psum = ps_mm.tile([128, 512], F32, tag="ymm")
                        for c in range(KC):
                            nc.tensor.matmul(
                                psum,
                                lhsT=wk_sb[:, c, j * 128 : (j + 1) * 128],
                                rhs=xk_sb[:, c, tb * 512 : (tb + 1) * 512],
                                start=(c == 0),
                                stop=(c == KC - 1),
                            )
                        nc.vector.tensor_copy(
                            kt_sb[0:64, 2 * j, tb * 512 : (tb + 1) * 512],
                            psum[0:64, :],
                        )
                        nc.vector.tensor_copy(
                            kt_sb[64:128, 2 * j + 1, tb * 512 : (tb + 1) * 512],
                            psum[64:128, :],
                        )
                        if j == 0:
                            psum = ps_o.tile([128, 512], F32, tag="o")
                        else:
                            psum = ps_mm.tile([128, 512], F32, tag="ymm")
                        for c in range(KC):
                            nc.tensor.matmul(
                                psum,
                                lhsT=wq_sb[:, c, j * 128 : (j + 1) * 128],
                                rhs=xq_sb[:, c, tb * 512 : (tb + 1) * 512],
                                start=(c == 0),
                                stop=(c == KC - 1),
                            )
                        # heads 2j (partitions 0:64) and 2j+1 (partitions 64:128)
                        nc.vector.tensor_scalar_add(
                            qt_sb[0:64, 2 * j, tb * 512 : (tb + 1) * 512],
                            psum[0:64, :],
                            bq_sb[0:64, j : j + 1],
                        )
                        nc.vector.tensor_scalar_add(
                            qt_sb[64:128, 2 * j + 1, tb * 512 : (tb + 1) * 512],
                            psum[64:128, :],
                            bq_sb[64:128, j : j + 1],
                        )

                # K staged first (scores need all of K^T but only one query
                # block), then Q; V and j0 projections fill PE meanwhile.
                stage_x(xk_sb, xkT, 0, nc.sync)
                stage_x(xq_sb, xqT, 0, nc.sync)
                emit_vproj(range(4))
                emit_qkproj(0)

            # --- attention
            if ablate:
                nc.gpsimd.memset(at_sb[:, :, :], 0.5)
                nc.gpsimd.memset(v_sb[:, :, :, :], 0.5)
            def emit_yproj(qt):
                y_sb = y_pool.tile([128, D], F32, tag="y")
                for nb in range(2):
                    y_psum = ps_o.tile([128, DG], F32, tag="o")
                    for fc in range(DG // 128):
                        nc.tensor.matmul(
                            y_psum,
                            lhsT=at_sb[:, fc, qt * 128 : (qt + 1) * 128],
                            rhs=wo_sb[:, fc, nb * DG : (nb + 1) * DG],
                            start=(fc == 0),
                            stop=(fc == DG // 128 - 1),
                        )
                    nc.vector.tensor_copy(y_sb[:, nb * DG : (nb + 1) * DG], y_psum)
                nc.sync.dma_start(out=y[qt * 128 : (qt + 1) * 128, :], in_=y_sb)

            pair_range = [] if "att" in ablate else range(3)
            for hp in pair_range:
                if hp > 0:
                    emit_qkproj(hp)
                for qb in range(QB):
                  for h in (2 * hp, 2 * hp + 1):
                    pt_t = pt_pool.tile([128, KT, 512], BF16, tag="pt")
                    for kt2 in range(KT // 2):
                        s_psum = ps_s.tile([128, 1024], F32, tag="s")
                        for half in range(2):
                            kt = 2 * kt2 + half
                            nc.tensor.matmul(
                                s_psum[:, half * 512 : (half + 1) * 512],
                                lhsT=kt_sb[:, h, kt * 128 : (kt + 1) * 128],
                                rhs=qt_sb[:, h, qb * 512 : (qb + 1) * 512],
                                start=True,
                                stop=True,
                            )
                        if "exp" not in ablate:
                            nc.scalar.activation(
                                pt_t[:, 2 * kt2 : 2 * kt2 + 2, :],
                                s_psum.rearrange("p (k f) -> p k f", k=2),
                                mybir.ActivationFunctionType.Exp,
                                scale=0.125,
                            )
                        else:
                            nc.gpsimd.memset(pt_t[:, 2 * kt2, 0:1], 0.5)
                    plo = 64 * (h % 2)
                    pv_range = [] if "pv" in ablate else range(4)
                    for qt in pv_range:
                        o_psum = ps_o.tile([128, DK + 1], F32, tag="o")
                        for kt in range(KT):
                            nc.tensor.matmul(
                                o_psum,
                                lhsT=pt_t[:, kt, qt * 128 : (qt + 1) * 128],
                                rhs=v_sb[:, kt, h, :],
                                start=(kt == 0),
                                stop=(kt == KT - 1),
                            )
                        recip = small_pool.tile([128, 1], F32, tag="recip")
                        nc.vector.reciprocal(recip, o_psum[:, DK : DK + 1])
                        attn = small_pool.tile([128, DK], BF16, tag="attn")
                        nc.vector.tensor_scalar_mul(attn, o_psum[:, 0:DK], recip)
                        if "tr" not in ablate:
                            t_psum = ps_t.tile([128, 128], BF16, tag="t")
                            nc.tensor.transpose(
                                t_psum[plo : plo + 64, :], attn, ident
                            )
                            nc.vector.tensor_copy(
                                at_sb[
                                    plo : plo + 64,
                                    h // 2,
                                    qb * 512 + qt * 128 : qb * 512 + (qt + 1) * 128,
                                ],
                                t_psum[plo : plo + 64, :],
                            )
                    if h == HG - 1 and "tr" not in ablate:
                        for qt4 in range(4):
                            emit_yproj(qb * 4 + qt4)

    _split_multi_waits(nc)
    return nc


_NC_CACHE: list = []


def _get_nc() -> bass.Bass:
    if not _NC_CACHE:
        _NC_CACHE.append(build_nc())
    return _NC_CACHE[0]


_RUNNER_CACHE: dict = {}


def _get_runner():
    """Compile once per process; later kernel() calls reuse the jitted
    executable (jax jit cache) and only re-upload inputs."""
    if _RUNNER_CACHE:
        return _RUNNER_CACHE["r"]
    import jax
    from jax.sharding import Mesh, PartitionSpec
    from jax.experimental.shard_map import shard_map
    from concourse import bass2jax

    nc = _get_nc()
    bass2jax.install_neuronx_cc_hook()
    partition_name = nc.partition_id_tensor.name if nc.partition_id_tensor else None
    in_names, out_names, out_avals, zero_outs = [], [], [], []
    for alloc in nc.m.functions[0].allocations:
        if not isinstance(alloc, mybir.MemoryLocationSet):
            continue
        name = alloc.memorylocations[0].name
        if alloc.kind == "ExternalInput":
            if name != partition_name:
                in_names.append(name)
        elif alloc.kind == "ExternalOutput":
            out_names.append(name)
            shape = tuple(alloc.tensor_shape)
            dtype = mybir.dt.np(alloc.dtype)
            out_avals.append(jax.core.ShapedArray(shape, dtype))
            zero_outs.append(np.zeros(shape, dtype))
    n_params = len(in_names)
    all_in_names = list(in_names) + list(out_names)
    if partition_name is not None:
        all_in_names.append(partition_name)

    def _body(*args):
        operands = list(args)
        if partition_name is not None:
            operands.append(bass2jax.partition_id_tensor())
        outs = bass2jax._bass_exec_p.bind(
            *operands,
            out_avals=tuple(out_avals),
            in_names=tuple(all_in_names),
            out_names=tuple(out_names),
            lowering_input_output_aliases=(),
            sim_require_finite=True,
            sim_require_nnan=True,
            nc=nc,
        )
        return tuple(outs)

    n_cores = 8
    devices = jax.devices()[:n_cores]
    mesh = Mesh(np.asarray(devices), ("core",))
    in_specs = (PartitionSpec("core"),) * (n_params + len(out_avals))
    out_specs = (PartitionSpec("core"),) * len(out_names)
    sharded = jax.jit(
        shard_map(_body, mesh=mesh, in_specs=in_specs, out_specs=out_specs,
                  check_rep=False),
        keep_unused=True,
    )
    concat_zeros = [
        np.zeros((n_cores * z.shape[0], *z.shape[1:]), z.dtype) for z in zero_outs
    ]

    import zlib

    dev_cache: dict = {}

    def run(in_maps):
        per_core = [[np.asarray(m[nm]) for nm in in_names] for m in in_maps]
        concat_in = [
            np.concatenate([per_core[c][i] for c in range(n_cores)], axis=0)
            for i in range(n_params)
        ]
        # Re-upload through the tunnel only when input content changes.
        key = tuple(
            (a.shape, str(a.dtype), zlib.adler32(a.tobytes())) for a in concat_in
        )
        if dev_cache.get("key") != key:
            dev_cache["args"] = [jax.device_put(a) for a in concat_in] + [
                jax.device_put(z) for z in concat_zeros
            ]
            dev_cache["key"] = key
        out = sharded(*dev_cache["args"])
        arrs = [np.asarray(x) for x in out]
        return [
            {
                nm: arrs[i].reshape(n_cores, *out_avals[i].shape)[c]
                for i, nm in enumerate(out_names)
            }
            for c in range(n_cores)
        ]

    _RUNNER_CACHE["r"] = run
    return run


def make_in_maps(q, k, v, Wq, bq, Wk, bk, Wv, bv, Wo, bo):
    q = np.asarray(q, np.float32)
    k = np.asarray(k, np.float32)
    v = np.asarray(v, np.float32)
    Wq = np.asarray(Wq, np.float32)
    Wk = np.asarray(Wk, np.float32)
    Wv = np.asarray(Wv, np.float32)
    Wo = np.asarray(Wo, np.float32)
    bq = np.asarray(bq, np.float32)

    xT = {}
    for b in range(B):
        xT[b] = (
            np.ascontiguousarray(q[b].T).astype(NPBF16),
            np.ascontiguousarray(k[b].T).astype(NPBF16),
            np.ascontiguousarray(v[b].T).astype(NPBF16),
        )
    wslice = {}
    for g in range(2):
        sl = slice(g * DG, (g + 1) * DG)
        wslice[g] = (
            np.ascontiguousarray(Wq[:, sl]).astype(NPBF16),
            np.ascontiguousarray(Wk[:, sl]).astype(NPBF16),
            np.ascontiguousarray(Wv[:, sl]).astype(NPBF16),
            np.ascontiguousarray(Wo[sl, :]).astype(NPBF16),
            np.ascontiguousarray(bq[sl]),
        )
    in_maps = []
    for c in range(8):
        b, g = c // 2, c % 2
        qT, kT, vT = xT[b]
        wq_g, wk_g, wv_g, wo_g, bq_g = wslice[g]
        in_maps.append(
            {
                "xqT": qT, "xkT": kT, "xvT": vT,
                "wq": wq_g, "wk": wk_g, "wv": wv_g, "wo": wo_g, "bq": bq_g,
            }
        )
    return in_maps


def gather(results, bv, bo, Wo):
    bv = np.asarray(bv, np.float32)
    bo = np.asarray(bo, np.float32)
    Wo = np.asarray(Wo, np.float32)
    const_row = bo + bv @ Wo  # [768]
    out = np.empty((B, N, D), np.float32)
    for b in range(B):
        out[b] = results[2 * b]["y"] + results[2 * b + 1]["y"] + const_row
    return out


def kernel(q, k, v, Wq, bq, Wk, bk, Wv, bv, Wo, bo):
    in_maps = make_in_maps(q, k, v, Wq, bq, Wk, bk, Wv, bv, Wo, bo)
    results = _get_runner()(in_maps)
    return gather(results, bv, bo, Wo)
